# revision 43
# baseline (speedup 1.0000x reference)
"""Causal single-head attention (B=4, S=2048, D=1024) on 8 Trainium2 cores.

Sharding: 8 cores = (batch b, stripe-set eta). Core (b, eta) owns eight
interleaved key stripes of 128 rows at global offsets 256k + 128*eta
(k = 0..7) of batch b, stored locally stripe-major. Queries are fed
"aligned" with base beta = 128*eta: query col c corresponds to global row
beta + c. The causal condition for key tile kt vs query chunk rc is
c >= 256*kt + x - identical on every core, so one SPMD program serves both
stripe sets with a compile-time block mask; score blocks with
kt >= 2*(rc+1) are skipped. Cols past the sequence end (eta=1, c >= 1920)
compute junk the host discards.

Softmax uses no max-subtraction (logits are O(1): |score/32| < ~1.8), so
per-core partials are num = exp(S)*V and l = sum(exp(S)); the host merges
halves and divides. Q and K projections fold into G = Wk^T Wq
(host-precomputed): scores = x_kv G x^T; M^T = G^T x_kv^T costs 1024*D^2,
replacing the 2048*D^2 Q proj + 1024*D^2 K proj.

Precision (gate max-rel-err < 2e-2; this scheme sims at 1.18e-2, and the
floor is entirely the fp8 scores path - fp8 far-V/PT adds nothing):
  fp8e4 DoubleRow (2x PE rate, contraction pairs via 3D APs [128,2,F]):
    MT, ST, V projection for key blocks 2..7, and all of PV for query
    chunks rc >= 1 (those rows see >= 512 keys, so V/PT quantization
    noise averages out).
  fp16: V projection for key blocks 0,1 and PV for rc = 0 - the
    few-visible-key rows where output ~= a single v row set the max err.
  G and Wv^T host-scaled by 32 (else their ~0.01-0.03 entries hit e4m3
  subnormals); exp scale absorbs G's 32, the host merge divides Wv's.
  num' <= ~4.4e3 so the numerator DMAs as fp16 (halves output traffic).
"""

import sys

sys.path.insert(0, "/opt/trn_rl_repo")

from contextlib import ExitStack

import ml_dtypes
import numpy as np

import concourse.bass as bass  # noqa: F401  (engine types resolve via bacc)
import concourse.mybir as mybir
import concourse.tile as tile
from concourse import bacc, bass_utils
from concourse.bass import ts

F8 = mybir.dt.float8e4
F16 = mybir.dt.float16
F32 = mybir.dt.float32
DR = mybir.MatmulPerfMode.DoubleRow
NP_F8 = ml_dtypes.float8_e4m3  # IEEE e4m3, max 240 — matches TRN FP8_EXP4

P = 128            # partitions
D = 1024           # model dim (d_in == d_out)
NQ = 2048          # query slots per core
NK = 1024          # keys per core
RC = 512           # query-chunk (matmul moving-dim) size
N_RC = NQ // RC    # 4
N_KT = NK // P     # 8 key tiles
N_IB = D // P      # 8 contraction blocks
N_PR = N_IB // 2   # 4 DoubleRow contraction pairs
SCALE8 = 1.0 / 1024.0  # exp scale: 1/sqrt(D) * 1/32 (G host-scale)

N_CORES = 8
B, S = 4, 2048
STRIPE = 128


def _kept_kts(rc):
    # key tile kt (128 keys at global 256*kt + 128*eta) is visible to
    # query chunk rc iff rc*512 + 511 >= 256*kt.
    return [kt for kt in range(N_KT) if kt < 2 * (rc + 1)]


def _mask_base(rc, kt):
    # stripe width 128: key tile kt IS stripe kt, threshold c >= 256*kt + x
    return RC * rc - 2 * P * kt


def _trim(rc, kt):
    # boundary tile kt == 2rc+1: its first 256 query cols lie strictly
    # below the causal diagonal.
    return 2 * P if kt == 2 * rc + 1 else 0


def _emit(nc, tc, xt8, xkv8, xkv16, g8, wv16, ot, ls):
    with ExitStack() as ctx:
        sb = ctx.enter_context(tc.tile_pool(name="sb", bufs=1))
        pts = ctx.enter_context(tc.tile_pool(name="pts", bufs=1))
        outp = ctx.enter_context(tc.tile_pool(name="outp", bufs=3))
        ps = ctx.enter_context(tc.tile_pool(name="ps", bufs=8, space="PSUM"))

        # all-ones STATIONARY blocks: the denominator reduction runs as a
        # single 512-col sweep per key tile whose [128, 512] output rows
        # all equal the column sums (full-width stationary keeps the PE
        # array config identical to the surrounding matmuls — a [*, 1]
        # output demonstrably drops the PE into a degraded mode)
        ones16 = sb.tile([P, P], F16, tag="ones16", name="ones16")
        nc.vector.memset(ones16, 1.0)
        ones8 = sb.tile([P, 2, P], F8, tag="ones8", name="ones8")
        nc.vector.memset(ones8, 1.0)

        # HAM warm-up: dummy matmuls needing no DMA, issued while the NEFF
        # preamble + first input loads run; lifts the PE clock gate from
        # 1.2 to 2.4 GHz. Result parked in l_sb (every column overwritten).
        warm = sb.tile([P, RC], F16, tag="warm", name="warm")
        nc.vector.memset(warm, 0.0)
        l_sb = sb.tile([1, NQ], F32, tag="lsb", name="lsb")
        acc_w = ps.tile([P, RC], F32, tag="mm", name="acc_w")
        N_WARM = 8
        for w in range(N_WARM):
            nc.tensor.matmul(acc_w, lhsT=warm[:, 0:P], rhs=warm,
                             start=(w == 0), stop=(w == N_WARM - 1))
        nc.vector.tensor_copy(warm[:, 0:16], acc_w[:, 0:16])

        # ---- input loads ----
        # All dram tensors are host-pre-arranged to the SBUF tile layout
        # [128, nblk, F], split into per-pair DMAs in consumption order:
        # MT needs g8+xkv8 only, so PE compute starts during the rest.
        g_sb = sb.tile([P, N_IB, D], F8, tag="g8", name="g8")
        xkv8_sb = sb.tile([P, N_IB, NK], F8, tag="xkv8", name="xkv8")
        xkv16_sb = sb.tile([P, N_IB, 2 * P], F16, tag="xkv16", name="xkv16")
        wv_sb = sb.tile([P, N_IB, D], F16, tag="wv16", name="wv16")
        wv8_sb = sb.tile([P, N_IB, D], F8, tag="wv8", name="wv8")
        xt_sb = sb.tile([P, N_IB, NQ], F8, tag="xt8", name="xt8")
        for q in range(N_PR):
            nc.sync.dma_start(out=g_sb[:, 2 * q:2 * q + 2, :],
                              in_=g8[:, 2 * q:2 * q + 2, :])
            nc.sync.dma_start(out=xkv8_sb[:, 2 * q:2 * q + 2, :],
                              in_=xkv8[:, 2 * q:2 * q + 2, :])
        nc.sync.dma_start(out=xkv16_sb, in_=xkv16)
        for q in range(N_PR):
            nc.sync.dma_start(out=wv_sb[:, 2 * q:2 * q + 2, :],
                              in_=wv16[:, 2 * q:2 * q + 2, :])
        for q in range(N_PR):
            nc.sync.dma_start(out=xt_sb[:, 2 * q:2 * q + 2, :],
                              in_=xt8[:, 2 * q:2 * q + 2, :])
        # wv8 is cast on-chip (saves 1MB of input DMA); scalar+vector are
        # both idle during the load phase
        for q in range(N_PR):
            eng = nc.scalar.activation if q % 2 else None
            if eng:
                eng(wv8_sb[:, 2 * q:2 * q + 2, :],
                    wv_sb[:, 2 * q:2 * q + 2, :],
                    mybir.ActivationFunctionType.Copy)
            else:
                nc.vector.tensor_copy(wv8_sb[:, 2 * q:2 * q + 2, :],
                                      wv_sb[:, 2 * q:2 * q + 2, :])

        # ---- MT projection (fp8 DoubleRow): M^T = g8^T xkv8 ----
        # batches of 8 PSUM groups: each contraction pair-step then spans
        # ~1.8us of matmuls, hiding the g8/xkv8 pair DMAs still in flight
        mt_sb = sb.tile([P, N_IB, NK], F8, tag="mt8", name="mt8")
        groups = [(o, jc) for o in range(N_IB) for jc in range(NK // RC)]
        for gb in range(0, len(groups), 8):
            batch = groups[gb:gb + 8]
            accs = [ps.tile([P, RC], F32, tag="mm", name="acc_mt")
                    for _ in batch]
            for q in range(N_PR):
                for a, (o, jc) in zip(accs, batch):
                    nc.tensor.matmul(
                        a, lhsT=g_sb[:, 2 * q:2 * q + 2, ts(o, P)],
                        rhs=xkv8_sb[:, 2 * q:2 * q + 2, ts(jc, RC)],
                        start=(q == 0), stop=(q == N_PR - 1), perf_mode=DR)
            for a, (o, jc) in zip(accs, batch):
                nc.vector.tensor_copy(mt_sb[:, o, ts(jc, RC)], a)

        # ---- V projection ----
        # Blocks 0,1 in fp16 (feed the precision-critical rc=0 diagonal);
        # the fp8 casts of those go via the scalar engine. Blocks 2..7 in
        # fp8 DoubleRow straight into v8.
        v16_sb = sb.tile([P, 2, D], F16, tag="v16", name="v16")
        v8_sb = sb.tile([P, N_KT, D], F8, tag="v8", name="v8")
        accs = [ps.tile([P, RC], F32, tag="mm", name="acc_v16")
                for _ in range(4)]
        for i in range(N_IB):
            for a, (j, dc) in zip(accs, [(0, 0), (0, 1), (1, 0), (1, 1)]):
                nc.tensor.matmul(a, lhsT=xkv16_sb[:, i, ts(j, P)],
                                 rhs=wv_sb[:, i, ts(dc, RC)],
                                 start=(i == 0), stop=(i == N_IB - 1))
        for a, (j, dc) in zip(accs, [(0, 0), (0, 1), (1, 0), (1, 1)]):
            nc.vector.tensor_copy(v16_sb[:, j, ts(dc, RC)], a)
            nc.scalar.activation(v8_sb[:, j, ts(dc, RC)], a,
                                 mybir.ActivationFunctionType.Copy)
        groups = [(j, dc) for j in range(2, N_KT) for dc in range(D // RC)]
        for gb in range(0, len(groups), 4):
            batch = groups[gb:gb + 4]
            accs = [ps.tile([P, RC], F32, tag="mm", name="acc_v8")
                    for _ in batch]
            for q in range(N_PR):
                for a, (j, dc) in zip(accs, batch):
                    nc.tensor.matmul(
                        a, lhsT=xkv8_sb[:, 2 * q:2 * q + 2, ts(j, P)],
                        rhs=wv8_sb[:, 2 * q:2 * q + 2, ts(dc, RC)],
                        start=(q == 0), stop=(q == N_PR - 1), perf_mode=DR)
            for a, (j, dc) in zip(accs, batch):
                nc.vector.tensor_copy(v8_sb[:, j, ts(dc, RC)], a)

        # ---- attention ----
        # ST emitted kt-major (stationary M^T block reused across rcs).
        # PT tiles: rc=0 in fp16; rc>=1 in fp8 pair layout [P, 2rc+2, RC]
        # (diagonal pair kt=2rc,2rc+1 included — it IS a DR pair).
        pt8 = {rc: pts.tile([P, 2 * rc + 2, RC], F8, tag=f"pt8_{rc}",
                            name=f"pt8_{rc}")
               for rc in range(1, N_RC)}
        # the odd diagonal tile's trimmed 256 cols are below-causal junk;
        # zero them once so the DR pair matmul can sweep them uniformly
        for rc in range(1, N_RC):
            nc.gpsimd.memset(pt8[rc][:, 2 * rc + 1, 0:2 * P], 0.0)
        pt16 = {}

        def emit_st(kt):
            rcs = [rc for rc in range(N_RC) if kt in _kept_kts(rc)]
            accs = {rc: ps.tile([P, RC], F32, tag="mm", name="acc_st")
                    for rc in rcs}
            for q in range(N_PR):
                for rc in rcs:
                    qo = _trim(rc, kt)
                    nc.tensor.matmul(
                        accs[rc][:, qo:RC],
                        lhsT=mt_sb[:, 2 * q:2 * q + 2, ts(kt, P)],
                        rhs=xt_sb[:, 2 * q:2 * q + 2,
                                  rc * RC + qo:(rc + 1) * RC],
                        start=(q == 0), stop=(q == N_PR - 1), perf_mode=DR)
            for rc in rcs:
                qo = _trim(rc, kt)
                base = _mask_base(rc, kt) + qo
                if rc == 0:          # precision-critical: fp16 + mask
                    pt = pts.tile([P, RC], F16, tag=f"pt16_{kt}",
                                  name=f"pt16_{kt}")
                    if qo:  # zero trimmed cols so the l row-sweep is exact
                        nc.gpsimd.memset(pt[:, 0:qo], 0.0)
                    nc.scalar.activation(pt[:, qo:RC], accs[rc][:, qo:RC],
                                         mybir.ActivationFunctionType.Exp,
                                         scale=SCALE8)
                    if base < P - 1:
                        nc.gpsimd.affine_select(
                            out=pt[:, qo:RC], in_=pt[:, qo:RC],
                            compare_op=mybir.AluOpType.is_ge, fill=0.0,
                            base=base, channel_multiplier=-1,
                            pattern=[[1, RC - qo]])
                    pt16[kt] = pt
                else:                # fp8 (far tiles never straddle)
                    dst = pt8[rc][:, kt, qo:RC]
                    nc.scalar.activation(dst, accs[rc][:, qo:RC],
                                         mybir.ActivationFunctionType.Exp,
                                         scale=SCALE8)
                    if base < P - 1:
                        nc.gpsimd.affine_select(
                            out=dst, in_=dst,
                            compare_op=mybir.AluOpType.is_ge, fill=0.0,
                            base=base, channel_multiplier=-1,
                            pattern=[[1, RC - qo]])

        def emit_lt(rc):
            # denominator: one full-width sweep per key tile; every output
            # row of lt equals the column sums (ones stationary)
            lt = ps.tile([P, RC], F32, tag="mm", name="lp")
            if rc == 0:
                for n, kt in enumerate((0, 1)):
                    nc.tensor.matmul(lt, lhsT=ones16, rhs=pt16[kt],
                                     start=(n == 0), stop=(n == 1))
            else:
                for t in range(rc + 1):
                    nc.tensor.matmul(lt, lhsT=ones8,
                                     rhs=pt8[rc][:, 2 * t:2 * t + 2, :],
                                     start=(t == 0), stop=(t == rc),
                                     perf_mode=DR)
            nc.vector.tensor_copy(l_sb[0:1, rc * RC:(rc + 1) * RC],
                                  lt[0:1, :])
            nc.sync.dma_start(out=ls[:, rc * RC:(rc + 1) * RC],
                              in_=l_sb[:, rc * RC:(rc + 1) * RC])

        def emit_pv(rc):
            if rc == N_RC - 1:   # keep the lt sweep off the critical tail
                emit_lt(rc)
            for rsub in range(RC // P):
                pos = [ps.tile([P, RC], F32, tag="mm", name="acc_pv")
                       for _ in range(D // RC)]
                if rc == 0:          # fp16 diagonal (kt 0 and trimmed 1)
                    kts = [kt for kt in (0, 1)
                           if not (rsub < _trim(0, kt) // P)]
                    for n, kt in enumerate(kts):
                        lhs = pt16[kt][:, ts(rsub, P)]
                        st, sp = (n == 0), (n == len(kts) - 1)
                        for dc, po in enumerate(pos):
                            nc.tensor.matmul(po, lhsT=lhs,
                                             rhs=v16_sb[:, kt, ts(dc, RC)],
                                             start=st, stop=sp)
                else:                # fp8 DoubleRow over all pairs
                    for t in range(rc + 1):
                        lhs = pt8[rc][:, 2 * t:2 * t + 2, ts(rsub, P)]
                        st, sp = (t == 0), (t == rc)
                        for dc, po in enumerate(pos):
                            nc.tensor.matmul(
                                po, lhsT=lhs,
                                rhs=v8_sb[:, 2 * t:2 * t + 2, ts(dc, RC)],
                                start=st, stop=sp, perf_mode=DR)
                # casts split across vector/scalar; one DMA per (rc,rsub)
                # for rc=3 (tail overlap), one per rc otherwise
                if rsub == 0:
                    emit_pv.o_big = outp.tile([P, RC // P, D], F16,
                                              tag="osb", name="osb")
                o_big = emit_pv.o_big
                nc.vector.tensor_copy(o_big[:, rsub, 0:RC], pos[0])
                nc.scalar.activation(o_big[:, rsub, RC:D], pos[1],
                                     mybir.ActivationFunctionType.Copy)
                if rc == N_RC - 1:
                    # pair (0,1); singles for 2 and 3 so the final
                    # transfer on the critical tail is only 256KB
                    if rsub == 1:
                        nc.sync.dma_start(out=ot[rc][:, 0:2, :],
                                          in_=o_big[:, 0:2, :])
                    elif rsub >= 2:
                        nc.sync.dma_start(out=ot[rc][:, rsub, :],
                                          in_=o_big[:, rsub, :])
                elif rsub == RC // P - 1:
                    nc.sync.dma_start(out=ot[rc], in_=o_big)
            if rc < N_RC - 1:
                emit_lt(rc)

        # software-pipelined emission: PV(rc) right after its last key tile
        emit_st(0)
        emit_st(1)
        emit_pv(0)
        emit_st(2)
        emit_st(3)
        emit_pv(1)
        emit_st(4)
        emit_st(5)
        emit_pv(2)
        emit_st(6)
        emit_st(7)
        emit_pv(3)


_NC_CACHE = {}


def _get_nc():
    if "nc" not in _NC_CACHE:
        nc = bacc.Bacc("TRN2", target_bir_lowering=False, debug=False,
                       enable_asserts=False, num_devices=N_CORES)
        xt8 = nc.dram_tensor("xt8", [P, N_IB, NQ], F8,
                             kind="ExternalInput").ap()
        xkv8 = nc.dram_tensor("xkv8", [P, N_IB, NK], F8,
                              kind="ExternalInput").ap()
        xkv16 = nc.dram_tensor("xkv16", [P, N_IB, 2 * P], F16,
                               kind="ExternalInput").ap()
        g8 = nc.dram_tensor("g8", [P, N_IB, D], F8, kind="ExternalInput").ap()
        wv16 = nc.dram_tensor("wv16", [P, N_IB, D], F16,
                              kind="ExternalInput").ap()
        # out rows interleave as row = rc*512 + rsub*128 + p
        ot = nc.dram_tensor("ot", [N_RC, P, RC // P, D], F16,
                            kind="ExternalOutput").ap()
        ls = nc.dram_tensor("ls", [1, NQ], F32, kind="ExternalOutput").ap()
        with tile.TileContext(nc) as tc:
            _emit(nc, tc, xt8, xkv8, xkv16, g8, wv16, ot, ls)
        nc.compile()
        _NC_CACHE["nc"] = nc
    return _NC_CACHE["nc"]


def _to_f8(a):
    return np.clip(a, -240.0, 240.0).astype(NP_F8)


def _blk(a):
    # [D, F] row-major -> SBUF layout [128, N_IB, F] (partition-major)
    return np.ascontiguousarray(a.reshape(N_IB, P, -1).transpose(1, 0, 2))


def make_in_maps(x, w_query, w_key, w_value):
    wq32 = np.asarray(w_query, dtype=np.float32)
    wk32 = np.asarray(w_key, dtype=np.float32)
    # fold the Q and K projections: scores = x_kv (Wk^T Wq) x^T.
    # 32x host scale keeps G's ~0.01 entries out of e4m3 subnormals; the
    # exp scale (1/1024) absorbs it.
    g_np = _blk(_to_f8(32.0 * (wk32.T @ wq32)))
    wvt32 = 32.0 * np.asarray(w_value, np.float32).T
    wv16_np = _blk(wvt32.astype(np.float16))
    kv_cols = (np.arange(NK) // STRIPE) * (2 * STRIPE) + np.arange(NK) % STRIPE
    in_maps = []
    for c in range(N_CORES):
        b, eta = c // 2, c % 2
        rows = (np.arange(NQ) + eta * STRIPE) % S  # cols past S wrap to junk
        xt_f32 = np.asarray(x)[b, rows].T          # [D, NQ]
        xt8_np = _to_f8(xt_f32)
        in_maps.append({
            "xt8": _blk(xt8_np),
            "xkv8": _blk(xt8_np[:, kv_cols]),
            "xkv16": _blk(xt_f32[:, kv_cols[:2 * P]].astype(np.float16)),
            "g8": g_np,
            "wv16": wv16_np,
        })
    return in_maps


def merge_outputs(results):
    num = np.zeros((B, S, D), np.float32)
    den = np.zeros((B, S), np.float32)
    for c in range(N_CORES):
        b, eta = c // 2, c % 2
        # ot rows interleave as [rc, p, rsub, d] -> row rc*512+rsub*128+p
        otc = np.asarray(results[c]["ot"]).astype(np.float32)
        otc = otc.transpose(0, 2, 1, 3).reshape(NQ, D)
        lc = np.asarray(results[c]["ls"]).reshape(NQ)
        beta = eta * STRIPE
        nvalid = S - beta
        num[b, beta:] += otc[:nvalid]
        den[b, beta:] += lc[:nvalid]
    # numerator carries the 32x Wv host scale
    return (num / (32.0 * den[:, :, None])).astype(np.float32)


def kernel(x, w_query, w_key, w_value, _trace=False):
    nc = _get_nc()
    in_maps = make_in_maps(x, w_query, w_key, w_value)
    res = bass_utils.run_bass_kernel_spmd(
        nc, in_maps, core_ids=list(range(N_CORES)), trace=_trace)
    out = merge_outputs(res.results)
    if _trace:
        kernel.last_result = res
    return out


# revision 44
# speedup vs baseline: 1.0154x; 1.0154x over previous
"""Causal single-head attention (B=4, S=2048, D=1024) on 8 Trainium2 cores.

Sharding: 8 cores = (batch b, stripe-set eta). Core (b, eta) owns eight
interleaved key stripes of 128 rows at global offsets 256k + 128*eta
(k = 0..7) of batch b, stored locally stripe-major. Queries are fed
"aligned" with base beta = 128*eta: query col c corresponds to global row
beta + c. The causal condition for key tile kt vs query chunk rc is
c >= 256*kt + x - identical on every core, so one SPMD program serves both
stripe sets with a compile-time block mask; score blocks with
kt >= 2*(rc+1) are skipped. Cols past the sequence end (eta=1, c >= 1920)
compute junk the host discards.

Softmax uses no max-subtraction (logits are O(1): |score/32| < ~1.8), so
per-core partials are num = exp(S)*V and l = sum(exp(S)); the host merges
halves and divides. Q and K projections fold into G = Wk^T Wq
(host-precomputed): scores = x_kv G x^T; M^T = G^T x_kv^T costs 1024*D^2,
replacing the 2048*D^2 Q proj + 1024*D^2 K proj.

Precision (gate max-rel-err < 2e-2; this scheme sims at 1.18e-2, and the
floor is entirely the fp8 scores path - fp8 far-V/PT adds nothing):
  fp8e4 DoubleRow (2x PE rate, contraction pairs via 3D APs [128,2,F]):
    MT, ST, V projection for key blocks 2..7, and all of PV for query
    chunks rc >= 1 (those rows see >= 512 keys, so V/PT quantization
    noise averages out).
  fp16: V projection for key blocks 0,1 and PV for rc = 0 - the
    few-visible-key rows where output ~= a single v row set the max err.
  G and Wv^T host-scaled by 32 (else their ~0.01-0.03 entries hit e4m3
  subnormals); exp scale absorbs G's 32, the host merge divides Wv's.
  num' <= ~4.4e3 so the numerator DMAs as fp16 (halves output traffic).
"""

import sys

sys.path.insert(0, "/opt/trn_rl_repo")

from contextlib import ExitStack

import ml_dtypes
import numpy as np

import concourse.bass as bass  # noqa: F401  (engine types resolve via bacc)
import concourse.mybir as mybir
import concourse.tile as tile
from concourse import bacc, bass_utils
from concourse.bass import ts

F8 = mybir.dt.float8e4
F16 = mybir.dt.float16
F32 = mybir.dt.float32
DR = mybir.MatmulPerfMode.DoubleRow
NP_F8 = ml_dtypes.float8_e4m3  # IEEE e4m3, max 240 — matches TRN FP8_EXP4

P = 128            # partitions
D = 1024           # model dim (d_in == d_out)
NQ = 2048          # query slots per core
NK = 1024          # keys per core
RC = 512           # query-chunk (matmul moving-dim) size
N_RC = NQ // RC    # 4
N_KT = NK // P     # 8 key tiles
N_IB = D // P      # 8 contraction blocks
N_PR = N_IB // 2   # 4 DoubleRow contraction pairs
SCALE8 = 1.0 / 1024.0  # exp scale: 1/sqrt(D) * 1/32 (G host-scale)

N_CORES = 8
B, S = 4, 2048
STRIPE = 128


def _kept_kts(rc):
    # key tile kt (128 keys at global 256*kt + 128*eta) is visible to
    # query chunk rc iff rc*512 + 511 >= 256*kt.
    return [kt for kt in range(N_KT) if kt < 2 * (rc + 1)]


def _mask_base(rc, kt):
    # stripe width 128: key tile kt IS stripe kt, threshold c >= 256*kt + x
    return RC * rc - 2 * P * kt


def _trim(rc, kt):
    # boundary tile kt == 2rc+1: its first 256 query cols lie strictly
    # below the causal diagonal.
    return 2 * P if kt == 2 * rc + 1 else 0


def _emit(nc, tc, xt8, xkv8, xkv16, g8, wv16, ot, ls):
    with ExitStack() as ctx:
        sb = ctx.enter_context(tc.tile_pool(name="sb", bufs=1))
        pts = ctx.enter_context(tc.tile_pool(name="pts", bufs=1))
        outp = ctx.enter_context(tc.tile_pool(name="outp", bufs=3))
        ps = ctx.enter_context(tc.tile_pool(name="ps", bufs=8, space="PSUM"))

        # all-ones STATIONARY blocks: the denominator reduction runs as a
        # single 512-col sweep per key tile whose [128, 512] output rows
        # all equal the column sums (full-width stationary keeps the PE
        # array config identical to the surrounding matmuls — a [*, 1]
        # output demonstrably drops the PE into a degraded mode)
        ones16 = sb.tile([P, P], F16, tag="ones16", name="ones16")
        nc.vector.memset(ones16, 1.0)
        ones8 = sb.tile([P, 2, P], F8, tag="ones8", name="ones8")
        nc.vector.memset(ones8, 1.0)

        # HAM warm-up: dummy matmuls needing no DMA, issued while the NEFF
        # preamble + first input loads run; lifts the PE clock gate from
        # 1.2 to 2.4 GHz. Result parked in l_sb (every column overwritten).
        warm = sb.tile([P, RC], F16, tag="warm", name="warm")
        nc.vector.memset(warm, 0.0)
        l_sb = sb.tile([1, NQ], F32, tag="lsb", name="lsb")
        acc_w = ps.tile([P, RC], F32, tag="mm", name="acc_w")
        N_WARM = 8
        for w in range(N_WARM):
            nc.tensor.matmul(acc_w, lhsT=warm[:, 0:P], rhs=warm,
                             start=(w == 0), stop=(w == N_WARM - 1))
        nc.vector.tensor_copy(warm[:, 0:16], acc_w[:, 0:16])

        # ---- input loads ----
        # All dram tensors are host-pre-arranged to the SBUF tile layout
        # [128, nblk, F], split into per-pair DMAs in consumption order:
        # MT needs g8+xkv8 only, so PE compute starts during the rest.
        g_sb = sb.tile([P, N_IB, D], F8, tag="g8", name="g8")
        xkv8_sb = sb.tile([P, N_IB, NK], F8, tag="xkv8", name="xkv8")
        xkv16_sb = sb.tile([P, N_IB, 2 * P], F16, tag="xkv16", name="xkv16")
        wv_sb = sb.tile([P, N_IB, D], F16, tag="wv16", name="wv16")
        wv8_sb = sb.tile([P, N_IB, D], F8, tag="wv8", name="wv8")
        xt_sb = sb.tile([P, N_IB, NQ], F8, tag="xt8", name="xt8")
        # g8 pairs split lo/hi: MT batch 0 (o-blocks 0..3) needs only the
        # lo halves, so the first compute window's DMA deps land ~1.4us
        # earlier; hi halves follow before MT batch 1 (~7us later)
        for q in range(N_PR):
            nc.sync.dma_start(out=g_sb[:, 2 * q:2 * q + 2, 0:RC],
                              in_=g8[:, 2 * q:2 * q + 2, 0:RC])
            nc.sync.dma_start(out=xkv8_sb[:, 2 * q:2 * q + 2, :],
                              in_=xkv8[:, 2 * q:2 * q + 2, :])
        for q in range(N_PR):
            nc.sync.dma_start(out=g_sb[:, 2 * q:2 * q + 2, RC:D],
                              in_=g8[:, 2 * q:2 * q + 2, RC:D])
        nc.sync.dma_start(out=xkv16_sb, in_=xkv16)
        for q in range(N_PR):
            nc.sync.dma_start(out=wv_sb[:, 2 * q:2 * q + 2, :],
                              in_=wv16[:, 2 * q:2 * q + 2, :])
        for q in range(N_PR):
            nc.sync.dma_start(out=xt_sb[:, 2 * q:2 * q + 2, :],
                              in_=xt8[:, 2 * q:2 * q + 2, :])
        # wv8 is cast on-chip (saves 1MB of input DMA); scalar+vector are
        # both idle during the load phase
        for q in range(N_PR):
            eng = nc.scalar.activation if q % 2 else None
            if eng:
                eng(wv8_sb[:, 2 * q:2 * q + 2, :],
                    wv_sb[:, 2 * q:2 * q + 2, :],
                    mybir.ActivationFunctionType.Copy)
            else:
                nc.vector.tensor_copy(wv8_sb[:, 2 * q:2 * q + 2, :],
                                      wv_sb[:, 2 * q:2 * q + 2, :])

        # ---- MT projection (fp8 DoubleRow): M^T = g8^T xkv8 ----
        # batches of 8 PSUM groups: each contraction pair-step then spans
        # ~1.8us of matmuls, hiding the g8/xkv8 pair DMAs still in flight
        mt_sb = sb.tile([P, N_IB, NK], F8, tag="mt8", name="mt8")
        groups = [(o, jc) for o in range(N_IB) for jc in range(NK // RC)]
        for gb in range(0, len(groups), 8):
            batch = groups[gb:gb + 8]
            accs = [ps.tile([P, RC], F32, tag="mm", name="acc_mt")
                    for _ in batch]
            for q in range(N_PR):
                for a, (o, jc) in zip(accs, batch):
                    nc.tensor.matmul(
                        a, lhsT=g_sb[:, 2 * q:2 * q + 2, ts(o, P)],
                        rhs=xkv8_sb[:, 2 * q:2 * q + 2, ts(jc, RC)],
                        start=(q == 0), stop=(q == N_PR - 1), perf_mode=DR)
            for a, (o, jc) in zip(accs, batch):
                nc.vector.tensor_copy(mt_sb[:, o, ts(jc, RC)], a)

        # ---- V projection ----
        # Blocks 0,1 in fp16 (feed the precision-critical rc=0 diagonal);
        # the fp8 casts of those go via the scalar engine. Blocks 2..7 in
        # fp8 DoubleRow straight into v8.
        v16_sb = sb.tile([P, 2, D], F16, tag="v16", name="v16")
        v8_sb = sb.tile([P, N_KT, D], F8, tag="v8", name="v8")
        accs = [ps.tile([P, RC], F32, tag="mm", name="acc_v16")
                for _ in range(4)]
        for i in range(N_IB):
            for a, (j, dc) in zip(accs, [(0, 0), (0, 1), (1, 0), (1, 1)]):
                nc.tensor.matmul(a, lhsT=xkv16_sb[:, i, ts(j, P)],
                                 rhs=wv_sb[:, i, ts(dc, RC)],
                                 start=(i == 0), stop=(i == N_IB - 1))
        for a, (j, dc) in zip(accs, [(0, 0), (0, 1), (1, 0), (1, 1)]):
            nc.vector.tensor_copy(v16_sb[:, j, ts(dc, RC)], a)
            nc.scalar.activation(v8_sb[:, j, ts(dc, RC)], a,
                                 mybir.ActivationFunctionType.Copy)
        groups = [(j, dc) for j in range(2, N_KT) for dc in range(D // RC)]
        for gb in range(0, len(groups), 4):
            batch = groups[gb:gb + 4]
            accs = [ps.tile([P, RC], F32, tag="mm", name="acc_v8")
                    for _ in batch]
            for q in range(N_PR):
                for a, (j, dc) in zip(accs, batch):
                    nc.tensor.matmul(
                        a, lhsT=xkv8_sb[:, 2 * q:2 * q + 2, ts(j, P)],
                        rhs=wv8_sb[:, 2 * q:2 * q + 2, ts(dc, RC)],
                        start=(q == 0), stop=(q == N_PR - 1), perf_mode=DR)
            for a, (j, dc) in zip(accs, batch):
                nc.vector.tensor_copy(v8_sb[:, j, ts(dc, RC)], a)

        # ---- attention ----
        # ST emitted kt-major (stationary M^T block reused across rcs).
        # PT tiles: rc=0 in fp16; rc>=1 in fp8 pair layout [P, 2rc+2, RC]
        # (diagonal pair kt=2rc,2rc+1 included — it IS a DR pair).
        pt8 = {rc: pts.tile([P, 2 * rc + 2, RC], F8, tag=f"pt8_{rc}",
                            name=f"pt8_{rc}")
               for rc in range(1, N_RC)}
        # the odd diagonal tile's trimmed 256 cols are below-causal junk;
        # zero them once so the DR pair matmul can sweep them uniformly
        for rc in range(1, N_RC):
            nc.gpsimd.memset(pt8[rc][:, 2 * rc + 1, 0:2 * P], 0.0)
        pt16 = {}

        def emit_st(kt):
            rcs = [rc for rc in range(N_RC) if kt in _kept_kts(rc)]
            accs = {rc: ps.tile([P, RC], F32, tag="mm", name="acc_st")
                    for rc in rcs}
            for q in range(N_PR):
                for rc in rcs:
                    qo = _trim(rc, kt)
                    nc.tensor.matmul(
                        accs[rc][:, qo:RC],
                        lhsT=mt_sb[:, 2 * q:2 * q + 2, ts(kt, P)],
                        rhs=xt_sb[:, 2 * q:2 * q + 2,
                                  rc * RC + qo:(rc + 1) * RC],
                        start=(q == 0), stop=(q == N_PR - 1), perf_mode=DR)
            for rc in rcs:
                qo = _trim(rc, kt)
                base = _mask_base(rc, kt) + qo
                if rc == 0:          # precision-critical: fp16 + mask
                    pt = pts.tile([P, RC], F16, tag=f"pt16_{kt}",
                                  name=f"pt16_{kt}")
                    if qo:  # zero trimmed cols so the l row-sweep is exact
                        nc.gpsimd.memset(pt[:, 0:qo], 0.0)
                    nc.scalar.activation(pt[:, qo:RC], accs[rc][:, qo:RC],
                                         mybir.ActivationFunctionType.Exp,
                                         scale=SCALE8)
                    if base < P - 1:
                        nc.gpsimd.affine_select(
                            out=pt[:, qo:RC], in_=pt[:, qo:RC],
                            compare_op=mybir.AluOpType.is_ge, fill=0.0,
                            base=base, channel_multiplier=-1,
                            pattern=[[1, RC - qo]])
                    pt16[kt] = pt
                else:                # fp8 (far tiles never straddle)
                    dst = pt8[rc][:, kt, qo:RC]
                    nc.scalar.activation(dst, accs[rc][:, qo:RC],
                                         mybir.ActivationFunctionType.Exp,
                                         scale=SCALE8)
                    if base < P - 1:
                        nc.gpsimd.affine_select(
                            out=dst, in_=dst,
                            compare_op=mybir.AluOpType.is_ge, fill=0.0,
                            base=base, channel_multiplier=-1,
                            pattern=[[1, RC - qo]])

        def emit_lt(rc):
            # denominator: one full-width sweep per key tile; every output
            # row of lt equals the column sums (ones stationary)
            lt = ps.tile([P, RC], F32, tag="mm", name="lp")
            if rc == 0:
                for n, kt in enumerate((0, 1)):
                    nc.tensor.matmul(lt, lhsT=ones16, rhs=pt16[kt],
                                     start=(n == 0), stop=(n == 1))
            else:
                for t in range(rc + 1):
                    nc.tensor.matmul(lt, lhsT=ones8,
                                     rhs=pt8[rc][:, 2 * t:2 * t + 2, :],
                                     start=(t == 0), stop=(t == rc),
                                     perf_mode=DR)
            nc.vector.tensor_copy(l_sb[0:1, rc * RC:(rc + 1) * RC],
                                  lt[0:1, :])
            nc.sync.dma_start(out=ls[:, rc * RC:(rc + 1) * RC],
                              in_=l_sb[:, rc * RC:(rc + 1) * RC])

        def emit_pv(rc):
            if rc == N_RC - 1:   # keep the lt sweep off the critical tail
                emit_lt(rc)
            for rsub in range(RC // P):
                pos = [ps.tile([P, RC], F32, tag="mm", name="acc_pv")
                       for _ in range(D // RC)]
                if rc == 0:          # fp16 diagonal (kt 0 and trimmed 1)
                    kts = [kt for kt in (0, 1)
                           if not (rsub < _trim(0, kt) // P)]
                    for n, kt in enumerate(kts):
                        lhs = pt16[kt][:, ts(rsub, P)]
                        st, sp = (n == 0), (n == len(kts) - 1)
                        for dc, po in enumerate(pos):
                            nc.tensor.matmul(po, lhsT=lhs,
                                             rhs=v16_sb[:, kt, ts(dc, RC)],
                                             start=st, stop=sp)
                else:                # fp8 DoubleRow over all pairs
                    for t in range(rc + 1):
                        lhs = pt8[rc][:, 2 * t:2 * t + 2, ts(rsub, P)]
                        st, sp = (t == 0), (t == rc)
                        for dc, po in enumerate(pos):
                            nc.tensor.matmul(
                                po, lhsT=lhs,
                                rhs=v8_sb[:, 2 * t:2 * t + 2, ts(dc, RC)],
                                start=st, stop=sp, perf_mode=DR)
                # casts split across vector/scalar; one DMA per (rc,rsub)
                # for rc=3 (tail overlap), one per rc otherwise
                if rsub == 0:
                    emit_pv.o_big = outp.tile([P, RC // P, D], F16,
                                              tag="osb", name="osb")
                o_big = emit_pv.o_big
                nc.vector.tensor_copy(o_big[:, rsub, 0:RC], pos[0])
                nc.scalar.activation(o_big[:, rsub, RC:D], pos[1],
                                     mybir.ActivationFunctionType.Copy)
                if rc == N_RC - 1:
                    # pair (0,1); singles for 2 and 3 so the final
                    # transfer on the critical tail is only 256KB
                    if rsub == 1:
                        nc.sync.dma_start(out=ot[rc][:, 0:2, :],
                                          in_=o_big[:, 0:2, :])
                    elif rsub >= 2:
                        nc.sync.dma_start(out=ot[rc][:, rsub, :],
                                          in_=o_big[:, rsub, :])
                elif rsub == RC // P - 1:
                    nc.sync.dma_start(out=ot[rc], in_=o_big)
            if rc < N_RC - 1:
                emit_lt(rc)

        # software-pipelined emission: PV(rc) right after its last key tile
        emit_st(0)
        emit_st(1)
        emit_pv(0)
        emit_st(2)
        emit_st(3)
        emit_pv(1)
        emit_st(4)
        emit_st(5)
        emit_pv(2)
        emit_st(6)
        emit_st(7)
        emit_pv(3)


_NC_CACHE = {}


def _get_nc():
    if "nc" not in _NC_CACHE:
        nc = bacc.Bacc("TRN2", target_bir_lowering=False, debug=False,
                       enable_asserts=False, num_devices=N_CORES)
        xt8 = nc.dram_tensor("xt8", [P, N_IB, NQ], F8,
                             kind="ExternalInput").ap()
        xkv8 = nc.dram_tensor("xkv8", [P, N_IB, NK], F8,
                              kind="ExternalInput").ap()
        xkv16 = nc.dram_tensor("xkv16", [P, N_IB, 2 * P], F16,
                               kind="ExternalInput").ap()
        g8 = nc.dram_tensor("g8", [P, N_IB, D], F8, kind="ExternalInput").ap()
        wv16 = nc.dram_tensor("wv16", [P, N_IB, D], F16,
                              kind="ExternalInput").ap()
        # out rows interleave as row = rc*512 + rsub*128 + p
        ot = nc.dram_tensor("ot", [N_RC, P, RC // P, D], F16,
                            kind="ExternalOutput").ap()
        ls = nc.dram_tensor("ls", [1, NQ], F32, kind="ExternalOutput").ap()
        with tile.TileContext(nc) as tc:
            _emit(nc, tc, xt8, xkv8, xkv16, g8, wv16, ot, ls)
        nc.compile()
        _NC_CACHE["nc"] = nc
    return _NC_CACHE["nc"]


def _to_f8(a):
    return np.clip(a, -240.0, 240.0).astype(NP_F8)


def _blk(a):
    # [D, F] row-major -> SBUF layout [128, N_IB, F] (partition-major)
    return np.ascontiguousarray(a.reshape(N_IB, P, -1).transpose(1, 0, 2))


def make_in_maps(x, w_query, w_key, w_value):
    wq32 = np.asarray(w_query, dtype=np.float32)
    wk32 = np.asarray(w_key, dtype=np.float32)
    # fold the Q and K projections: scores = x_kv (Wk^T Wq) x^T.
    # 32x host scale keeps G's ~0.01 entries out of e4m3 subnormals; the
    # exp scale (1/1024) absorbs it.
    g_np = _blk(_to_f8(32.0 * (wk32.T @ wq32)))
    wvt32 = 32.0 * np.asarray(w_value, np.float32).T
    wv16_np = _blk(wvt32.astype(np.float16))
    kv_cols = (np.arange(NK) // STRIPE) * (2 * STRIPE) + np.arange(NK) % STRIPE
    in_maps = []
    for c in range(N_CORES):
        b, eta = c // 2, c % 2
        rows = (np.arange(NQ) + eta * STRIPE) % S  # cols past S wrap to junk
        xt_f32 = np.asarray(x)[b, rows].T          # [D, NQ]
        xt8_np = _to_f8(xt_f32)
        in_maps.append({
            "xt8": _blk(xt8_np),
            "xkv8": _blk(xt8_np[:, kv_cols]),
            "xkv16": _blk(xt_f32[:, kv_cols[:2 * P]].astype(np.float16)),
            "g8": g_np,
            "wv16": wv16_np,
        })
    return in_maps


def merge_outputs(results):
    num = np.zeros((B, S, D), np.float32)
    den = np.zeros((B, S), np.float32)
    for c in range(N_CORES):
        b, eta = c // 2, c % 2
        # ot rows interleave as [rc, p, rsub, d] -> row rc*512+rsub*128+p
        otc = np.asarray(results[c]["ot"]).astype(np.float32)
        otc = otc.transpose(0, 2, 1, 3).reshape(NQ, D)
        lc = np.asarray(results[c]["ls"]).reshape(NQ)
        beta = eta * STRIPE
        nvalid = S - beta
        num[b, beta:] += otc[:nvalid]
        den[b, beta:] += lc[:nvalid]
    # numerator carries the 32x Wv host scale
    return (num / (32.0 * den[:, :, None])).astype(np.float32)


def kernel(x, w_query, w_key, w_value, _trace=False):
    nc = _get_nc()
    in_maps = make_in_maps(x, w_query, w_key, w_value)
    res = bass_utils.run_bass_kernel_spmd(
        nc, in_maps, core_ids=list(range(N_CORES)), trace=_trace)
    out = merge_outputs(res.results)
    if _trace:
        kernel.last_result = res
    return out
